# revision 24
# baseline (speedup 1.0000x reference)
"""Trainium2 Bass kernel for AdditiveUnpoolingWrapper.

  proj_down = gelu(LN(down @ W_down + b_down))          [M, 128]
  proj_skip = gelu(LN(residual @ W_skip + b_skip))      [N, 128]
  out       = proj_skip + proj_down[subbuck_idx]        [N, 128]

Sharding strategy (8 cores, all compute on device):
  The pooled-bucket space M=262144 is split into 8 contiguous ranges of
  32768 rows; core i owns range i and computes that slice of proj_down
  into a 16 MB local DRAM table. Points (rows of residual) are assigned
  to the core that owns their subbuck_idx — i.e. data-parallel over
  points with a bucket-aligned assignment — so the gather is local to
  the core's own table and local indices fit in [0, 32768). The host
  sorts points by subbuck_idx (shards become contiguous, and in-shard
  gathers hit ascending addresses), pads each shard to a common CAP,
  and inverse-permutes the concatenated device outputs back to the
  original point order.  Weights are replicated.

Device kernel notes:
  - LayerNorm is fused into the gelu ACTIVATE via per-partition
    scale/bias (scale=rstd, bias=-mu*rstd), so the ACT engine runs a
    single table set (gelu) for the whole kernel — no ~2.7us
    ACT_TABLE_LOAD switches.
  - rstd = rsqrt(var+eps) is computed on the Vector engine with the
    bit-trick seed + 3 Newton steps (max rel err ~1.5e-7), batched
    across a group of SGRP chunks to amortize per-op overhead.
  - The gather uses the GPSIMD dma_gather ucode (mlp library), one
    2048-row gather per group to amortize the ~1us SWDGE fixed cost.
"""

import numpy as np

N = 524288
M = 262144
C_IN = 256
C_SKIP = 128
C_OUT = 128
LN_EPS = 1e-5
NCORES = 8
SH = M // NCORES  # table rows per core (32768)
P = 128
GRP = 4  # 128-point matmul groups per chunk
CHUNK = P * GRP  # points per chunk (512); one PSUM bank
SGRP = 4  # chunks per group (batched stats / one gather per group)
GPTS = CHUNK * SGRP  # points per group (2048)
SG = SGRP * GRP  # 128-pt tiles per group (16)
RSQRT_MAGIC = 0x5F3759DF

_PROG_CACHE = {}


def _wrap_idx_i16(li, cap):
    """dma_gather index layout: index i lives at partition i%16, free i//16,
    replicated across the 8 gpsimd cores (partition blocks of 16)."""
    w = li.astype(np.int16).reshape(cap // 16, 16).T
    return np.ascontiguousarray(np.tile(w, (8, 1)))


def _build_program(cap, dn_rows, trivial_params):
    """Build + compile the SPMD Bass program.

    cap      : padded points per core (multiple of GPTS)
    dn_rows  : down/table rows per core (multiple of GPTS)
    trivial_params : True when b_down/b_skip are 0 and ln_g/ln_b are 1/0
                     (lets us skip the per-free-element affine ops).
    """
    from contextlib import ExitStack

    import concourse.bass as bass
    import concourse.tile as tile
    from bass_rust import add_dep_helper
    from concourse import bacc, library_config, mybir

    f32 = mybir.dt.float32
    i16 = mybir.dt.int16
    i32 = mybir.dt.int32
    AF = mybir.ActivationFunctionType
    ALU = mybir.AluOpType

    assert cap % GPTS == 0 and dn_rows % GPTS == 0

    nc = bacc.Bacc("TRN2", target_bir_lowering=False, debug=False,
                   num_devices=NCORES)

    down_t = nc.dram_tensor("down_t", [C_IN, dn_rows], f32, kind="ExternalInput").ap()
    resid_t = nc.dram_tensor("resid_t", [C_SKIP, cap], f32, kind="ExternalInput").ap()
    idxw = nc.dram_tensor("idxw", [P, cap // 16], i16, kind="ExternalInput").ap()
    w_down = nc.dram_tensor("w_down", [C_IN, C_OUT], f32, kind="ExternalInput").ap()
    w_skip = nc.dram_tensor("w_skip", [C_SKIP, C_OUT], f32, kind="ExternalInput").ap()
    # packed per-channel params: [b_down, g_down, bl_down, b_skip, g_skip, bl_skip]
    params = nc.dram_tensor("params", [6, C_OUT], f32, kind="ExternalInput").ap()
    table = nc.dram_tensor("table", [dn_rows, C_OUT], f32, kind="Internal").ap()
    out = nc.dram_tensor("out", [cap, C_OUT], f32, kind="ExternalOutput").ap()

    kd = C_IN // P  # 2 k-chunks for the down projection

    with tile.TileContext(nc) as tc, ExitStack() as ctx:
        consts = ctx.enter_context(tc.tile_pool(name="consts", bufs=1))
        a_in = ctx.enter_context(tc.tile_pool(name="a_in", bufs=3))
        a_out = ctx.enter_context(tc.tile_pool(name="a_out", bufs=3))
        a_psum = ctx.enter_context(tc.tile_pool(name="a_psum", bufs=4, space="PSUM"))
        b_in = ctx.enter_context(tc.tile_pool(name="b_in", bufs=3))
        b_out = ctx.enter_context(tc.tile_pool(name="b_out", bufs=3))
        b_psum = ctx.enter_context(tc.tile_pool(name="b_psum", bufs=4, space="PSUM"))
        stats = ctx.enter_context(tc.tile_pool(name="stats", bufs=4))

        # ---- constants ----
        wd = consts.tile([P, kd, C_OUT], f32, tag="wd")
        nc.sync.dma_start(wd[:], w_down.rearrange("(a p) n -> p a n", p=P))
        ws = consts.tile([P, C_OUT], f32, tag="ws")
        nc.sync.dma_start(ws[:], w_skip[:, :])
        magic_t = consts.tile([P, SG], i32, tag="magic")
        nc.vector.memset(magic_t[:], RSQRT_MAGIC)
        idx_sb = consts.tile([P, cap // 16], i16, tag="idx")
        nc.sync.dma_start(idx_sb[:], idxw[:, :])
        with tc.tile_critical():
            nc.gpsimd.load_library(library_config.mlp)

        if not trivial_params:
            # broadcast per-channel params across all 128 partitions
            par_sb = consts.tile([P, 6, C_OUT], f32, tag="par")
            par_bcast = bass.AP(
                tensor=params.tensor,
                offset=params.offset,
                ap=[[0, P], params.ap[0], params.ap[1]],
            )
            nc.sync.dma_start(par_sb[:], par_bcast)

        def group_stats_start():
            return (stats.tile([P, SG, 6], f32, tag="bn", name="st"),
                    stats.tile([P, SG, 2], f32, tag="mv", name="mv"))

        def chunk_stats(psum, mv, st, cc, bias_idx):
            """bn stats for one chunk's [P, CHUNK] psum into mv[:, cc*GRP+g]."""
            if not trivial_params:
                psum3 = psum[:].rearrange("p (g c) -> p g c", g=GRP)
                nc.vector.tensor_tensor(
                    out=psum3, in0=psum3,
                    in1=par_sb[:, bias_idx:bias_idx + 1, :].to_broadcast(
                        [P, GRP, C_OUT]),
                    op=ALU.add)
            for g in range(GRP):
                j = cc * GRP + g
                nc.vector.bn_stats(st[:, j, :], psum[:, g * C_OUT:(g + 1) * C_OUT])
                nc.vector.bn_aggr(mv[:, j, :], st[:, j, :])

        def group_rstd(mv):
            """Batched rstd = rsqrt(var+eps) and nbias = -mu*rstd on DVE."""
            v = stats.tile([P, SG], f32, tag="v")
            rstd = stats.tile([P, SG], f32, tag="rstd")
            tmp = stats.tile([P, SG], f32, tag="tmp")
            nbias = stats.tile([P, SG], f32, tag="nbias")
            nc.vector.tensor_scalar(out=v[:], in0=mv[:, :, 1], scalar1=LN_EPS,
                                    scalar2=None, op0=ALU.add)
            v_i = v[:].bitcast(i32)
            r_i = rstd[:].bitcast(i32)
            nc.vector.tensor_scalar(out=r_i, in0=v_i, scalar1=1, scalar2=None,
                                    op0=ALU.logical_shift_right)
            nc.vector.tensor_tensor(out=r_i, in0=magic_t[:], in1=r_i,
                                    op=ALU.subtract)
            for _ in range(3):
                nc.vector.tensor_tensor(out=tmp[:], in0=rstd[:], in1=rstd[:],
                                        op=ALU.mult)
                nc.vector.tensor_tensor(out=tmp[:], in0=v[:], in1=tmp[:],
                                        op=ALU.mult)
                nc.vector.tensor_scalar(out=tmp[:], in0=tmp[:], scalar1=-0.5,
                                        scalar2=1.5, op0=ALU.mult, op1=ALU.add)
                nc.vector.tensor_tensor(out=rstd[:], in0=rstd[:], in1=tmp[:],
                                        op=ALU.mult)
            nc.vector.tensor_tensor(out=nbias[:], in0=mv[:, :, 0], in1=rstd[:],
                                    op=ALU.mult)
            nc.vector.tensor_scalar(out=nbias[:], in0=nbias[:], scalar1=-1.0,
                                    scalar2=None, op0=ALU.mult)
            return rstd, nbias

        def chunk_act(psum, rstd, nbias, cc, dest, g_idx, bl_idx):
            """gelu(LN(x)) from psum into dest[:, cc*GRP+g, :] slices."""
            if trivial_params:
                for g in range(GRP):
                    j = cc * GRP + g
                    nc.scalar.activation(
                        dest[:, j, :], psum[:, g * C_OUT:(g + 1) * C_OUT],
                        AF.Gelu_apprx_tanh,
                        bias=nbias[:, j:j + 1], scale=rstd[:, j:j + 1])
            else:
                xn = stats.tile([P, GRP, C_OUT], f32, tag="xn")
                for g in range(GRP):
                    j = cc * GRP + g
                    nc.scalar.activation(
                        xn[:, g, :], psum[:, g * C_OUT:(g + 1) * C_OUT],
                        AF.Identity,
                        bias=nbias[:, j:j + 1], scale=rstd[:, j:j + 1])
                nc.vector.tensor_tensor(
                    out=xn[:], in0=xn[:],
                    in1=par_sb[:, g_idx:g_idx + 1, :].to_broadcast([P, GRP, C_OUT]),
                    op=ALU.mult)
                nc.vector.tensor_tensor(
                    out=xn[:], in0=xn[:],
                    in1=par_sb[:, bl_idx:bl_idx + 1, :].to_broadcast([P, GRP, C_OUT]),
                    op=ALU.add)
                sl3 = dest[:, cc * GRP:(cc + 1) * GRP, :]
                nc.scalar.activation(sl3, xn[:], AF.Gelu_apprx_tanh)

        # ---- phase A: build this core's slice of proj_down ----
        table_writes = []
        down3 = down_t.rearrange("(a p) n -> p a n", p=P)
        with nc.named_scope("phaseA"):
            for gi_ in range(dn_rows // GPTS):
                go = gi_ * GPTS
                dtile = a_in.tile([P, kd, GPTS], f32, tag="dtile")
                nc.sync.dma_start(dtile[:], down3[:, :, go:go + GPTS])
                st, mv = group_stats_start()
                psums = []
                for cc in range(SGRP):
                    psum = a_psum.tile([P, CHUNK], f32, tag="apsum")
                    psums.append(psum)
                    for g in range(GRP):
                        sl = slice((cc * GRP + g) * P, (cc * GRP + g + 1) * P)
                        for a in range(kd):
                            nc.tensor.matmul(
                                out=psum[:, g * P:(g + 1) * P],
                                lhsT=dtile[:, a, sl], rhs=wd[:, a, :],
                                start=(a == 0), stop=(a == kd - 1))
                    chunk_stats(psum, mv, st, cc, 0)
                rstd, nbias = group_rstd(mv)
                ptile = a_out.tile([P, SG, C_OUT], f32, tag="ptile")
                for cc in range(SGRP):
                    chunk_act(psums[cc], rstd, nbias, cc, ptile, 1, 2)
                w = nc.sync.dma_start(
                    table[go:go + GPTS, :].rearrange("(g p) c -> p g c", p=P),
                    ptile[:])
                table_writes.append(w)

        # join node: all table writes complete (DRAM RAW deps between DMAs
        # are not tracked automatically, so make the gathers wait explicitly)
        table_ready = nc.sync.nop()
        for w in table_writes:
            add_dep_helper(table_ready.ins, w.ins,
                           reason="table_ready waits on table write")

        # ---- phase B: skip projection + gather + add ----
        with nc.named_scope("phaseB"):
            for gi_ in range(cap // GPTS):
                go = gi_ * GPTS
                rtile = b_in.tile([P, GPTS], f32, tag="rtile")
                nc.sync.dma_start(rtile[:], resid_t[:, go:go + GPTS])
                # gather for the whole group; dma_gather tops out at 1024
                # indices per call, so issue GPTS//1024 calls
                gtile = b_out.tile([P, SG, C_OUT], f32, tag="gtile")
                gnum = 1024
                for h in range(GPTS // gnum):
                    ho = go + h * gnum
                    gath = nc.gpsimd.dma_gather(
                        gtile[:, h * (gnum // P):(h + 1) * (gnum // P), :],
                        table[:, :],
                        idx_sb[:, ho // 16:(ho + gnum) // 16],
                        gnum, gnum, C_OUT)
                    add_dep_helper(gath.ins, table_ready.ins,
                                   reason="gather waits on table_ready")
                st, mv = group_stats_start()
                psums = []
                for cc in range(SGRP):
                    psum = b_psum.tile([P, CHUNK], f32, tag="bpsum")
                    psums.append(psum)
                    for g in range(GRP):
                        sl = slice((cc * GRP + g) * P, (cc * GRP + g + 1) * P)
                        nc.tensor.matmul(out=psum[:, g * P:(g + 1) * P],
                                         lhsT=rtile[:, sl], rhs=ws[:, :],
                                         start=True, stop=True)
                    chunk_stats(psum, mv, st, cc, 3)
                rstd, nbias = group_rstd(mv)
                stile = b_out.tile([P, SG, C_OUT], f32, tag="stile")
                for cc in range(SGRP):
                    chunk_act(psums[cc], rstd, nbias, cc, stile, 4, 5)
                nc.vector.tensor_tensor(out=stile[:], in0=stile[:],
                                        in1=gtile[:], op=ALU.add)
                nc.sync.dma_start(
                    out[go:go + GPTS, :].rearrange("(g p) c -> p g c", p=P),
                    stile[:])

    nc.compile()
    return nc


def _get_program(cap, dn_rows, trivial_params):
    key = (cap, dn_rows, trivial_params)
    if key not in _PROG_CACHE:
        _PROG_CACHE[key] = _build_program(cap, dn_rows, trivial_params)
    return _PROG_CACHE[key]


def kernel(residual, down, W_down, b_down, ln_g_down, ln_b_down,
           W_skip, b_skip, ln_g_skip, ln_b_skip, subbuck_idx):
    from concourse.bass_utils import run_bass_kernel_spmd

    residual = np.ascontiguousarray(np.asarray(residual, dtype=np.float32))
    down = np.ascontiguousarray(np.asarray(down, dtype=np.float32))
    W_down = np.ascontiguousarray(np.asarray(W_down, dtype=np.float32))
    W_skip = np.ascontiguousarray(np.asarray(W_skip, dtype=np.float32))
    idx = np.asarray(subbuck_idx).astype(np.int32)
    pvecs = [np.asarray(v, dtype=np.float32) for v in
             (b_down, ln_g_down, ln_b_down, b_skip, ln_g_skip, ln_b_skip)]
    trivial = (not pvecs[0].any() and not pvecs[3].any()
               and np.all(pvecs[1] == 1) and np.all(pvecs[4] == 1)
               and not pvecs[2].any() and not pvecs[5].any())
    params = np.stack(pvecs).astype(np.float32)

    n = idx.shape[0]
    assert residual.shape == (n, C_SKIP) and down.shape == (M, C_IN)

    # ---- host-side sharding: sort points by bucket ----
    order = np.argsort(idx, kind="stable")
    sorted_idx = idx[order]
    bounds = np.searchsorted(sorted_idx, np.arange(NCORES + 1) * SH)
    counts = np.diff(bounds)
    cap = int(np.ceil(max(counts.max(), 1) / GPTS) * GPTS)

    nc = _get_program(cap, SH, trivial)

    down_T = np.ascontiguousarray(down.T)  # [C_IN, M]
    in_maps = []
    segs = []
    for i in range(NCORES):
        seg = order[bounds[i]:bounds[i + 1]]
        segs.append(seg)
        ni = seg.shape[0]
        rt = np.zeros((cap, C_SKIP), np.float32)
        rt[:ni] = residual[seg]
        li = np.zeros(cap, np.int32)
        li[:ni] = sorted_idx[bounds[i]:bounds[i + 1]] - i * SH
        in_maps.append({
            "down_t": np.ascontiguousarray(down_T[:, i * SH:(i + 1) * SH]),
            "resid_t": np.ascontiguousarray(rt.T),
            "idxw": _wrap_idx_i16(li, cap),
            "w_down": W_down,
            "w_skip": W_skip,
            "params": params,
        })

    global _LAST_RUN
    _LAST_RUN = (nc, in_maps)
    res = run_bass_kernel_spmd(nc, in_maps, core_ids=list(range(NCORES)))

    out = np.empty((n, C_OUT), np.float32)
    for i in range(NCORES):
        out[segs[i]] = res.results[i]["out"][:segs[i].shape[0]]
    return out


# revision 29
# speedup vs baseline: 1.0005x; 1.0005x over previous
"""Trainium2 Bass kernel for AdditiveUnpoolingWrapper.

  proj_down = gelu(LN(down @ W_down + b_down))          [M, 128]
  proj_skip = gelu(LN(residual @ W_skip + b_skip))      [N, 128]
  out       = proj_skip + proj_down[subbuck_idx]        [N, 128]

Sharding strategy (8 cores, all compute on device):
  The pooled-bucket space M=262144 is split into 8 contiguous ranges of
  32768 rows; core i owns range i and computes that slice of proj_down
  into a 16 MB local DRAM table. Points (rows of residual) are assigned
  to the core that owns their subbuck_idx — i.e. data-parallel over
  points with a bucket-aligned assignment — so the gather is local to
  the core's own table and local indices fit in [0, 32768). The host
  sorts points by subbuck_idx (shards become contiguous, and in-shard
  gathers hit ascending addresses), pads each shard to a common CAP,
  and inverse-permutes the concatenated device outputs back to the
  original point order.  Weights are replicated.

Device kernel notes:
  - LayerNorm is fused into the gelu ACTIVATE via per-partition
    scale/bias (scale=rstd, bias=-mu*rstd), so the ACT engine runs a
    single table set (gelu) for the whole kernel — no ~2.7us
    ACT_TABLE_LOAD switches.
  - rstd = rsqrt(var+eps) is computed on the Vector engine with the
    bit-trick seed + 3 Newton steps (max rel err ~1.5e-7), batched
    across a group of SGRP chunks to amortize per-op overhead.
  - The gather uses the GPSIMD dma_gather ucode (mlp library), one
    2048-row gather per group to amortize the ~1us SWDGE fixed cost.
"""

import numpy as np

N = 524288
M = 262144
C_IN = 256
C_SKIP = 128
C_OUT = 128
LN_EPS = 1e-5
NCORES = 8
SH = M // NCORES  # table rows per core (32768)
P = 128
GRP = 4  # 128-point matmul groups per chunk
CHUNK = P * GRP  # points per chunk (512); one PSUM bank
SGRP = 4  # chunks per group (batched stats / one gather per group)
GPTS = CHUNK * SGRP  # points per group (2048)
SG = SGRP * GRP  # 128-pt tiles per group (16)
RSQRT_MAGIC = 0x5F3759DF

_PROG_CACHE = {}


def _wrap_idx_i16(li, cap):
    """dma_gather index layout: index i lives at partition i%16, free i//16,
    replicated across the 8 gpsimd cores (partition blocks of 16)."""
    w = li.astype(np.int16).reshape(cap // 16, 16).T
    return np.ascontiguousarray(np.tile(w, (8, 1)))


GNUM = 1024  # indices per dma_gather call (2048 crashes the SWDGE ucode)


def _build_program(cap, dn_rows, trivial_params, gdeps=None):
    """Build + compile the SPMD Bass program.

    cap      : padded points per core (multiple of GPTS)
    dn_rows  : down/table rows per core (multiple of GPTS)
    trivial_params : True when b_down/b_skip are 0 and ln_g/ln_b are 1/0
                     (lets us skip the per-free-element affine ops).
    gdeps    : per gather call (cap//GNUM entries), the highest phase-A
               table group (dn_rows//GPTS groups) whose rows that call can
               touch, maxed across all cores.  Lets each gather start as
               soon as its prefix of the table is written instead of
               waiting for all of phase A.  None -> all calls wait for the
               full table.
    """
    from contextlib import ExitStack

    import concourse.bass as bass
    import concourse.tile as tile
    from bass_rust import add_dep_helper
    from concourse import bacc, library_config, mybir

    f32 = mybir.dt.float32
    i16 = mybir.dt.int16
    i32 = mybir.dt.int32
    AF = mybir.ActivationFunctionType
    ALU = mybir.AluOpType

    assert cap % GPTS == 0 and dn_rows % GPTS == 0

    nc = bacc.Bacc("TRN2", target_bir_lowering=False, debug=False,
                   num_devices=NCORES)

    down_t = nc.dram_tensor("down_t", [C_IN, dn_rows], f32, kind="ExternalInput").ap()
    resid_t = nc.dram_tensor("resid_t", [C_SKIP, cap], f32, kind="ExternalInput").ap()
    idxw = nc.dram_tensor("idxw", [P, cap // 16], i16, kind="ExternalInput").ap()
    w_down = nc.dram_tensor("w_down", [C_IN, C_OUT], f32, kind="ExternalInput").ap()
    w_skip = nc.dram_tensor("w_skip", [C_SKIP, C_OUT], f32, kind="ExternalInput").ap()
    # packed per-channel params: [b_down, g_down, bl_down, b_skip, g_skip, bl_skip]
    params = nc.dram_tensor("params", [6, C_OUT], f32, kind="ExternalInput").ap()
    table = nc.dram_tensor("table", [dn_rows, C_OUT], f32, kind="Internal").ap()
    out = nc.dram_tensor("out", [cap, C_OUT], f32, kind="ExternalOutput").ap()

    kd = C_IN // P  # 2 k-chunks for the down projection

    with tile.TileContext(nc) as tc, ExitStack() as ctx:
        consts = ctx.enter_context(tc.tile_pool(name="consts", bufs=1))
        a_in = ctx.enter_context(tc.tile_pool(name="a_in", bufs=3))
        a_out = ctx.enter_context(tc.tile_pool(name="a_out", bufs=3))
        a_psum = ctx.enter_context(tc.tile_pool(name="a_psum", bufs=4, space="PSUM"))
        b_in = ctx.enter_context(tc.tile_pool(name="b_in", bufs=3))
        b_out = ctx.enter_context(tc.tile_pool(name="b_out", bufs=3))
        b_psum = ctx.enter_context(tc.tile_pool(name="b_psum", bufs=4, space="PSUM"))
        stats = ctx.enter_context(tc.tile_pool(name="stats", bufs=4))

        # ---- constants ----
        wd = consts.tile([P, kd, C_OUT], f32, tag="wd")
        nc.sync.dma_start(wd[:], w_down.rearrange("(a p) n -> p a n", p=P))
        ws = consts.tile([P, C_OUT], f32, tag="ws")
        nc.sync.dma_start(ws[:], w_skip[:, :])
        magic_t = consts.tile([P, SG], i32, tag="magic")
        nc.vector.memset(magic_t[:], RSQRT_MAGIC)
        idx_sb = consts.tile([P, cap // 16], i16, tag="idx")
        nc.sync.dma_start(idx_sb[:], idxw[:, :])
        with tc.tile_critical():
            nc.gpsimd.load_library(library_config.mlp)

        if not trivial_params:
            # broadcast per-channel params across all 128 partitions
            par_sb = consts.tile([P, 6, C_OUT], f32, tag="par")
            par_bcast = bass.AP(
                tensor=params.tensor,
                offset=params.offset,
                ap=[[0, P], params.ap[0], params.ap[1]],
            )
            nc.sync.dma_start(par_sb[:], par_bcast)

        def group_stats_start():
            return (stats.tile([P, SG, 6], f32, tag="bn", name="st"),
                    stats.tile([P, SG, 2], f32, tag="mv", name="mv"))

        def chunk_stats(psum, mv, st, cc, bias_idx):
            """bn stats for one chunk's [P, CHUNK] psum into mv[:, cc*GRP+g]."""
            if not trivial_params:
                psum3 = psum[:].rearrange("p (g c) -> p g c", g=GRP)
                nc.vector.tensor_tensor(
                    out=psum3, in0=psum3,
                    in1=par_sb[:, bias_idx:bias_idx + 1, :].to_broadcast(
                        [P, GRP, C_OUT]),
                    op=ALU.add)
            for g in range(GRP):
                j = cc * GRP + g
                nc.vector.bn_stats(st[:, j, :], psum[:, g * C_OUT:(g + 1) * C_OUT])
                nc.vector.bn_aggr(mv[:, j, :], st[:, j, :])

        def group_rstd(mv):
            """Batched rstd = rsqrt(var+eps) and nbias = -mu*rstd on DVE."""
            v = stats.tile([P, SG], f32, tag="v")
            rstd = stats.tile([P, SG], f32, tag="rstd")
            tmp = stats.tile([P, SG], f32, tag="tmp")
            nbias = stats.tile([P, SG], f32, tag="nbias")
            nc.vector.tensor_scalar(out=v[:], in0=mv[:, :, 1], scalar1=LN_EPS,
                                    scalar2=None, op0=ALU.add)
            v_i = v[:].bitcast(i32)
            r_i = rstd[:].bitcast(i32)
            nc.vector.tensor_scalar(out=r_i, in0=v_i, scalar1=1, scalar2=None,
                                    op0=ALU.logical_shift_right)
            nc.vector.tensor_tensor(out=r_i, in0=magic_t[:], in1=r_i,
                                    op=ALU.subtract)
            for _ in range(3):
                nc.vector.tensor_tensor(out=tmp[:], in0=rstd[:], in1=rstd[:],
                                        op=ALU.mult)
                nc.vector.tensor_tensor(out=tmp[:], in0=v[:], in1=tmp[:],
                                        op=ALU.mult)
                nc.vector.tensor_scalar(out=tmp[:], in0=tmp[:], scalar1=-0.5,
                                        scalar2=1.5, op0=ALU.mult, op1=ALU.add)
                nc.vector.tensor_tensor(out=rstd[:], in0=rstd[:], in1=tmp[:],
                                        op=ALU.mult)
            nc.vector.tensor_tensor(out=nbias[:], in0=mv[:, :, 0], in1=rstd[:],
                                    op=ALU.mult)
            nc.vector.tensor_scalar(out=nbias[:], in0=nbias[:], scalar1=-1.0,
                                    scalar2=None, op0=ALU.mult)
            return rstd, nbias

        def chunk_act(psum, rstd, nbias, cc, dest, g_idx, bl_idx):
            """gelu(LN(x)) from psum into dest[:, cc*GRP+g, :] slices."""
            if trivial_params:
                for g in range(GRP):
                    j = cc * GRP + g
                    nc.scalar.activation(
                        dest[:, j, :], psum[:, g * C_OUT:(g + 1) * C_OUT],
                        AF.Gelu_apprx_tanh,
                        bias=nbias[:, j:j + 1], scale=rstd[:, j:j + 1])
            else:
                xn = stats.tile([P, GRP, C_OUT], f32, tag="xn")
                for g in range(GRP):
                    j = cc * GRP + g
                    nc.scalar.activation(
                        xn[:, g, :], psum[:, g * C_OUT:(g + 1) * C_OUT],
                        AF.Identity,
                        bias=nbias[:, j:j + 1], scale=rstd[:, j:j + 1])
                nc.vector.tensor_tensor(
                    out=xn[:], in0=xn[:],
                    in1=par_sb[:, g_idx:g_idx + 1, :].to_broadcast([P, GRP, C_OUT]),
                    op=ALU.mult)
                nc.vector.tensor_tensor(
                    out=xn[:], in0=xn[:],
                    in1=par_sb[:, bl_idx:bl_idx + 1, :].to_broadcast([P, GRP, C_OUT]),
                    op=ALU.add)
                sl3 = dest[:, cc * GRP:(cc + 1) * GRP, :]
                nc.scalar.activation(sl3, xn[:], AF.Gelu_apprx_tanh)

        # ---- phase A: build this core's slice of proj_down ----
        table_writes = []
        down3 = down_t.rearrange("(a p) n -> p a n", p=P)
        with nc.named_scope("phaseA"):
            for gi_ in range(dn_rows // GPTS):
                go = gi_ * GPTS
                dtile = a_in.tile([P, kd, GPTS], f32, tag="dtile")
                nc.sync.dma_start(dtile[:], down3[:, :, go:go + GPTS])
                st, mv = group_stats_start()
                psums = []
                for cc in range(SGRP):
                    psum = a_psum.tile([P, CHUNK], f32, tag="apsum")
                    psums.append(psum)
                    for g in range(GRP):
                        sl = slice((cc * GRP + g) * P, (cc * GRP + g + 1) * P)
                        for a in range(kd):
                            nc.tensor.matmul(
                                out=psum[:, g * P:(g + 1) * P],
                                lhsT=dtile[:, a, sl], rhs=wd[:, a, :],
                                start=(a == 0), stop=(a == kd - 1))
                    chunk_stats(psum, mv, st, cc, 0)
                rstd, nbias = group_rstd(mv)
                ptile = a_out.tile([P, SG, C_OUT], f32, tag="ptile")
                for cc in range(SGRP):
                    chunk_act(psums[cc], rstd, nbias, cc, ptile, 1, 2)
                w = nc.sync.dma_start(
                    table[go:go + GPTS, :].rearrange("(g p) c -> p g c", p=P),
                    ptile[:])
                table_writes.append(w)

        # per-group join chain: ready[g] = table groups 0..g written (DRAM
        # RAW deps between DMAs are not tracked automatically, so gathers
        # wait on these explicitly)
        ready = []
        for g, w in enumerate(table_writes):
            nop = nc.sync.nop()
            add_dep_helper(nop.ins, w.ins,
                           reason=f"table group {g} written")
            if ready:
                add_dep_helper(nop.ins, ready[-1].ins,
                               reason="chain previous table groups")
            ready.append(nop)
        n_tbl_groups = len(table_writes)
        if gdeps is None:
            gdeps = (n_tbl_groups - 1,) * (cap // GNUM)
        assert len(gdeps) == cap // GNUM
        assert all(0 <= d < n_tbl_groups for d in gdeps)

        # ---- phase B: skip projection + gather + add ----
        with nc.named_scope("phaseB"):
            for gi_ in range(cap // GPTS):
                go = gi_ * GPTS
                rtile = b_in.tile([P, GPTS], f32, tag="rtile")
                nc.sync.dma_start(rtile[:], resid_t[:, go:go + GPTS])
                # gather for the whole group; dma_gather tops out at 1024
                # indices per call, so issue GPTS//GNUM calls
                gtile = b_out.tile([P, SG, C_OUT], f32, tag="gtile")
                for h in range(GPTS // GNUM):
                    ho = go + h * GNUM
                    gath = nc.gpsimd.dma_gather(
                        gtile[:, h * (GNUM // P):(h + 1) * (GNUM // P), :],
                        table[:, :],
                        idx_sb[:, ho // 16:(ho + GNUM) // 16],
                        GNUM, GNUM, C_OUT)
                    add_dep_helper(gath.ins, ready[gdeps[ho // GNUM]].ins,
                                   reason="gather waits on its table prefix")
                st, mv = group_stats_start()
                psums = []
                for cc in range(SGRP):
                    psum = b_psum.tile([P, CHUNK], f32, tag="bpsum")
                    psums.append(psum)
                    for g in range(GRP):
                        sl = slice((cc * GRP + g) * P, (cc * GRP + g + 1) * P)
                        nc.tensor.matmul(out=psum[:, g * P:(g + 1) * P],
                                         lhsT=rtile[:, sl], rhs=ws[:, :],
                                         start=True, stop=True)
                    chunk_stats(psum, mv, st, cc, 3)
                rstd, nbias = group_rstd(mv)
                stile = b_out.tile([P, SG, C_OUT], f32, tag="stile")
                for cc in range(SGRP):
                    chunk_act(psums[cc], rstd, nbias, cc, stile, 4, 5)
                nc.vector.tensor_tensor(out=stile[:], in0=stile[:],
                                        in1=gtile[:], op=ALU.add)
                nc.sync.dma_start(
                    out[go:go + GPTS, :].rearrange("(g p) c -> p g c", p=P),
                    stile[:])

    nc.compile()
    return nc


def _get_program(cap, dn_rows, trivial_params, gdeps=None):
    key = (cap, dn_rows, trivial_params, gdeps)
    if key not in _PROG_CACHE:
        _PROG_CACHE[key] = _build_program(cap, dn_rows, trivial_params, gdeps)
    return _PROG_CACHE[key]


def kernel(residual, down, W_down, b_down, ln_g_down, ln_b_down,
           W_skip, b_skip, ln_g_skip, ln_b_skip, subbuck_idx):
    from concourse.bass_utils import run_bass_kernel_spmd

    residual = np.ascontiguousarray(np.asarray(residual, dtype=np.float32))
    down = np.ascontiguousarray(np.asarray(down, dtype=np.float32))
    W_down = np.ascontiguousarray(np.asarray(W_down, dtype=np.float32))
    W_skip = np.ascontiguousarray(np.asarray(W_skip, dtype=np.float32))
    idx = np.asarray(subbuck_idx).astype(np.int32)
    pvecs = [np.asarray(v, dtype=np.float32) for v in
             (b_down, ln_g_down, ln_b_down, b_skip, ln_g_skip, ln_b_skip)]
    trivial = (not pvecs[0].any() and not pvecs[3].any()
               and np.all(pvecs[1] == 1) and np.all(pvecs[4] == 1)
               and not pvecs[2].any() and not pvecs[5].any())
    params = np.stack(pvecs).astype(np.float32)

    n = idx.shape[0]
    assert residual.shape == (n, C_SKIP) and down.shape == (M, C_IN)

    # ---- host-side sharding: sort points by bucket ----
    order = np.argsort(idx, kind="stable")
    sorted_idx = idx[order]
    bounds = np.searchsorted(sorted_idx, np.arange(NCORES + 1) * SH)
    counts = np.diff(bounds)
    cap = int(np.ceil(max(counts.max(), 1) / GPTS) * GPTS)

    down_T = np.ascontiguousarray(down.T)  # [C_IN, M]
    in_maps = []
    segs = []
    lis = []
    for i in range(NCORES):
        seg = order[bounds[i]:bounds[i + 1]]
        segs.append(seg)
        ni = seg.shape[0]
        rt = np.zeros((cap, C_SKIP), np.float32)
        rt[:ni] = residual[seg]
        li = np.zeros(cap, np.int32)
        li[:ni] = sorted_idx[bounds[i]:bounds[i + 1]] - i * SH
        lis.append(li)
        in_maps.append({
            "down_t": np.ascontiguousarray(down_T[:, i * SH:(i + 1) * SH]),
            "resid_t": np.ascontiguousarray(rt.T),
            "idxw": _wrap_idx_i16(li, cap),
            "w_down": W_down,
            "w_skip": W_skip,
            "params": params,
        })

    # per-gather-call table-group dependency, maxed across cores (indices
    # are sorted per core, so each 1024-point block only needs a prefix of
    # the table)
    li_all = np.stack(lis).reshape(NCORES, cap // GNUM, GNUM)
    gdeps = tuple(int(d) for d in li_all.max(axis=(0, 2)) // GPTS)

    nc = _get_program(cap, SH, trivial, gdeps)

    global _LAST_RUN
    _LAST_RUN = (nc, in_maps)
    res = run_bass_kernel_spmd(nc, in_maps, core_ids=list(range(NCORES)))

    out = np.empty((n, C_OUT), np.float32)
    for i in range(NCORES):
        out[segs[i]] = res.results[i]["out"][:segs[i].shape[0]]
    return out
